# revision 10
# baseline (speedup 1.0000x reference)
"""Embedding-similarity group merge on 8 Trainium2 NeuronCores.

Strategy
--------
The heavy part of the reference (Embeddings._fast_predict) is the blocked
cosine-similarity score computation V @ V.T (16384 x 16384 x 256 ~ 137 GFLOP).
The transitive group-merge that follows is inherently sequential and
path-dependent (final labels are NOT canonical connected-component ids), but
it only touches the ~4-5k above-threshold pairs, so it is cheap on host.

Device: V row-block per core, bf16 matmul (full PE rate) producing a uint8
candidate mask  (sims_bf16 >= thr - EPS).  With both operands rounded to
bf16, |sims_bf16 - sims_fp32| <= 2^-8 + accumulation noise << EPS = 0.01,
so the mask is a guaranteed superset of the true fp32-threshold matches.

Host: gathers candidate pairs, recomputes their sims exactly in fp32,
applies the reference's column mask (j >= batch_start+1), and replays the
reference's sequential batch/row merge to produce bit-identical group ids.
"""

import sys

if "/opt/trn_rl_repo" not in sys.path:
    sys.path.insert(0, "/opt/trn_rl_repo")

import numpy as np
import ml_dtypes

import concourse.bass as bass
import concourse.tile as tile
from concourse import bacc, mybir
from concourse.bass_utils import run_bass_kernel_spmd

N_CORES = 8
D = 256                     # embedding dim (2 chunks of 128 on partitions)
EPS = 0.01                  # bf16 guard band (worst-case bf16 error ~0.004)
I_TILE = 128                # psum partition tile (query rows per matmul)
J_TILE = 512                # psum free-dim tile (one full psum bank, fp32)
J_GROUP = 4                 # j-tiles per output staging DMA (256KB contiguous)

_BUILD_CACHE: dict = {}
LAST_EXEC_NS = None         # set when kernel() runs with TRACE=True
TRACE = False


def _ensure_ntff_hook():
    """Register the axon NTFF-profile hook (test/trace path only).

    The agent image's ``antenv`` lacks ``axon_hooks``, so ``trn_boot.boot``
    silently skips hook registration and ``bass_utils`` would crash on the
    import. Seed ``sys.modules['antenv.axon_hooks']`` with a stub wired to
    the ctypes hook so ``trace=True`` yields real NTFF profiles."""
    import types
    if "antenv.axon_hooks" in sys.modules:
        return
    try:
        from trn_agent_boot.trn_boot import _ntff_profile_via_ctypes
        hook = _ntff_profile_via_ctypes("/opt/axon/libaxon_pjrt.so")
    except Exception:
        hook = None
    mod = types.ModuleType("antenv.axon_hooks")
    mod._HOOK = hook
    mod.get_axon_ntff_profile_hook = lambda: mod._HOOK
    mod.set_axon_ntff_profile_hook = lambda h: setattr(mod, "_HOOK", h)
    sys.modules["antenv.axon_hooks"] = mod


def _build_program(n_cols: int, n_rows: int, thr_dev: float) -> bass.Bass:
    """One SPMD program: per-core candidate mask for an n_rows block of
    queries against all n_cols database vectors.

    Inputs (per core):
      vt [2, 128, n_cols] bf16 -- V.T split into two 128-row d-chunks
      vq [2, 128, n_rows] bf16 -- this core's query columns of V.T
    Output:
      out [n_rows, n_cols] uint8 -- 1 where sims_bf16 >= thr_dev
    """
    nc = bacc.Bacc(None, target_bir_lowering=False)
    vt_d = nc.declare_dram_parameter("vt", [2, 128, n_cols], mybir.dt.bfloat16, isOutput=False)
    vq_d = nc.declare_dram_parameter("vq", [2, 128, n_rows], mybir.dt.bfloat16, isOutput=False)

    n_itiles = n_rows // I_TILE
    n_jtiles = n_cols // J_TILE
    jg_tiles = min(J_GROUP, n_jtiles)
    n_jgroups = n_jtiles // jg_tiles
    # [t, jg, p, c] layout: each (t, jg) staging tile lands contiguous.
    out_d = nc.declare_dram_parameter(
        "out", [n_itiles, n_jgroups, I_TILE, jg_tiles * J_TILE],
        mybir.dt.uint8, isOutput=True)

    with tile.TileContext(nc) as tc:
        with (
            tc.tile_pool(name="vt", bufs=1) as vt_pool,
            tc.tile_pool(name="vq", bufs=1) as vq_pool,
            tc.tile_pool(name="psum", bufs=8, space="PSUM") as psum_pool,
            tc.tile_pool(name="stage", bufs=3) as stage_pool,
        ):
            vt_sb = vt_pool.tile([128, 2, n_cols], mybir.dt.bfloat16)
            vq_sb = vq_pool.tile([128, 2, n_rows], mybir.dt.bfloat16)
            for c in range(2):
                nc.sync.dma_start(out=vt_sb[:, c, :], in_=vt_d[c])
                nc.sync.dma_start(out=vq_sb[:, c, :], in_=vq_d[c])

            for t in range(n_itiles):
                ts = slice(t * I_TILE, (t + 1) * I_TILE)
                for jg in range(n_jgroups):
                    stage = stage_pool.tile([128, jg_tiles * J_TILE], mybir.dt.uint8)
                    for jj in range(jg_tiles):
                        j = jg * jg_tiles + jj
                        ps = psum_pool.tile([128, J_TILE], mybir.dt.float32)
                        js = slice(j * J_TILE, (j + 1) * J_TILE)
                        nc.tensor.matmul(
                            ps, lhsT=vq_sb[:, 0, ts], rhs=vt_sb[:, 0, js],
                            start=True, stop=False,
                        )
                        nc.tensor.matmul(
                            ps, lhsT=vq_sb[:, 1, ts], rhs=vt_sb[:, 1, js],
                            start=False, stop=True,
                        )
                        nc.vector.tensor_scalar(
                            stage[:, jj * J_TILE:(jj + 1) * J_TILE], ps,
                            thr_dev, None, mybir.AluOpType.is_ge
                        )
                    nc.gpsimd.dma_start(out=out_d[t, jg], in_=stage)
    nc.finalize()
    return nc


def _device_candidate_mask(V32: np.ndarray, thr: float) -> np.ndarray:
    """Run the SPMD kernel on 8 cores; return full [N, N] uint8 mask of
    candidates (sims_bf16 >= thr - EPS)."""
    global LAST_EXEC_NS
    n = V32.shape[0]
    rows = n // N_CORES
    thr_dev = float(thr) - EPS

    key = (n, rows, round(thr_dev, 9))
    if key not in _BUILD_CACHE:
        _BUILD_CACHE[key] = _build_program(n, rows, thr_dev)
    nc = _BUILD_CACHE[key]

    vt16 = np.ascontiguousarray(
        V32.T.reshape(2, 128, n).astype(ml_dtypes.bfloat16)
    )
    in_maps = []
    for c in range(N_CORES):
        vq16 = np.ascontiguousarray(vt16[:, :, c * rows:(c + 1) * rows])
        in_maps.append({"vt": vt16, "vq": vq16})

    if TRACE:
        _ensure_ntff_hook()
    res = run_bass_kernel_spmd(
        nc, in_maps, core_ids=list(range(N_CORES)), trace=TRACE
    )
    if TRACE:
        LAST_EXEC_NS = res.exec_time_ns
    blocks = []
    for c in range(N_CORES):
        o = res.results[c]["out"]  # [n_itiles, n_jgroups, I_TILE, w]
        blocks.append(o.transpose(0, 2, 1, 3).reshape(rows, n))
    return np.concatenate(blocks, axis=0)


def _exact_edges(V32, mask, thr, B):
    """From the device candidate mask, produce exact reference edges:
    fp32 sims >= thr and j >= (i//B)*B + 1.  Returns (ci, cj) sorted by i."""
    ii, jj = np.nonzero(mask)
    # column mask of the reference: j >= batch_start + 1
    keep = jj >= (ii // B) * B + 1
    ii, jj = ii[keep], jj[keep]
    if ii.size:
        sims = np.einsum("ij,ij->i", V32[ii], V32[jj])
        keep = sims >= np.float32(thr)
        ii, jj = ii[keep], jj[keep]
    return ii.astype(np.int64), jj.astype(np.int64)


def _merge_replay(g, ci, cj, B):
    """Faithful replay of the reference's sequential merge.

    Per batch: the matched sets are frozen at batch start (with the
    g_i0 != g_j filter evaluated on batch-start group ids), then rows are
    processed sequentially; each row i merges every row whose CURRENT group
    id appears among the CURRENT group ids of its matched j's into i's
    CURRENT group."""
    n = g.shape[0]
    if ci.size == 0:
        return g
    order = np.argsort(ci, kind="stable")
    ci, cj = ci[order], cj[order]
    # row -> slice of cj
    row_ids, row_starts = np.unique(ci, return_index=True)
    row_ends = np.append(row_starts[1:], ci.size)
    row_j = {int(i): cj[s:e] for i, s, e in zip(row_ids, row_starts, row_ends)}

    flag = np.zeros(max(n, int(g.max()) + 1), dtype=bool)
    for b in np.unique(row_ids // B):
        bs = int(b) * B
        g0 = g.copy()
        frozen = []
        for i in range(bs, bs + B):
            J = row_j.get(i)
            if J is None:
                continue
            J = J[g0[J] != g0[i]]
            if J.size:
                frozen.append((i, J))
        for i, J in frozen:
            mg = np.unique(g[J])
            flag[mg] = True
            sel = flag[g]
            g[sel] = g[i]
            flag[mg] = False
    return g


def kernel(V, group_ids, cos_threshold, batch_size):
    V32 = np.ascontiguousarray(np.asarray(V, dtype=np.float32))
    g = np.asarray(group_ids, dtype=np.int32).copy()
    thr = float(np.asarray(cos_threshold).reshape(-1)[0])
    B = int(np.asarray(batch_size))

    mask = _device_candidate_mask(V32, thr)
    ci, cj = _exact_edges(V32, mask, thr, B)
    g = _merge_replay(g, ci, cj, B)
    return g.astype(np.int32)
